# revision 29
# baseline (speedup 1.0000x reference)
"""Trainium2 Bass kernel for an AttentionBlock (GroupNorm + 8-head attention
over 32x32 spatial + proj + residual), data-parallel over batch on 8 cores.

Per batch element (x: [C=512, n=1024]):
  h   = GroupNorm32(x) * scale + bias          (h stored fp8, channel-packed)
  q,k = Wq h, Wk h  (bf16; k-bias provably drops out of softmax; q-bias kept)
  v^T = h^T Wv^T    (fp8, n-major, DoubleRow-packed, + ones row for Z)
  S^T = k^T q / 8   per head (bf16 matmuls, two heads packed in PE quadrants)
  e   = exp(S/8 - 2.5)  -> fp8   [ACT engine big chunks + DVE Schraudolph share]
  o   = v e (fp8 DoubleRow), Z from the ones row; att = o / Z
  y   = x + Wp att + pb_eff      (pb_eff = pb + Wp @ vb, folded host-side)

All convs run fp8 DoubleRow with channel-pair packing (c = 256t + 128j + p).
PSUM is managed as one 8-bank arena of four [128,1024] half-slots.
"""

import numpy as np

import concourse.bacc as bacc
import concourse.bass as bass
import concourse.tile as tile
from concourse import mybir
from concourse.bass_utils import run_bass_kernel_spmd

F32 = mybir.dt.float32
I32 = mybir.dt.int32
BF16 = mybir.dt.bfloat16
FP8 = mybir.dt.float8e4
AF = mybir.ActivationFunctionType
OP = mybir.AluOpType

C = 512
NH = 8
D = 64
N = 1024
GROUPS = 32
GS = C // GROUPS
EPS = 1e-5
B_PER_CORE = 2
N_CORES = 8

CT = 4    # channel tiles of 128
NT = 8    # n tiles of 128
VG = 66   # vT per-head group stride (64 data + 1 ones + 1 pad)

LOG2E = 1.4426950408889634
EXP_SHIFT = -2.5          # softmax-invariant logit shift keeps e in fp8 range
# Schraudolph exp in bf16 bit domain: int16 bits = round(S*A16 + B16) is the
# bf16 pattern of ~exp(0.125*S + EXP_SHIFT) (3% approx err; softmax
# normalization cancels the common-mode part). A@V reads the bits via bitcast.
SCHRA_A16 = float((1 << 7) * LOG2E * 0.125)
SCHRA_B16 = float((1 << 7) * (127.0 - 0.0450466 + EXP_SHIFT * LOG2E))

# e-tile pairs (of n-tiles) per head-pair whose exp runs on DVE instead of ACT
DVE_PAIRS = {0: (1,), 1: (1,), 2: (1,), 3: (1,)}
I16 = mybir.dt.int16
N_WARM_BOUNDARY = 8   # warm-keeper dummy matmuls after each AV block
N_WARM_PROLOGUE = 16


def build_nc(has_qb, has_pb, trace_sim=False):
    nc = bacc.Bacc()

    x_ext = nc.declare_dram_parameter("x", [B_PER_CORE, 128, CT, N], F32, isOutput=False)
    w_ext = {}
    for nm in ("q", "k", "v", "p"):
        w_ext[nm] = nc.declare_dram_parameter(f"{nm}w8", [128, 2, 2, C], FP8, isOutput=False)
    # packed per-channel vectors: [128, 4, CT] = (norm_scale, norm_bias, qb, pb_eff)
    vecs_ext = nc.declare_dram_parameter("vecs", [128, 4, CT], F32, isOutput=False)
    selr_ext = nc.declare_dram_parameter("selr", [128, CT, GROUPS], BF16, isOutput=False)
    sele_ext = nc.declare_dram_parameter("sele", [GROUPS, CT, 128], BF16, isOutput=False)
    out_ext = nc.declare_dram_parameter("out", [B_PER_CORE, 128, CT, N], F32, isOutput=True)

    zdram = nc.dram_tensor("zscratch", [B_PER_CORE, NH, N], F32)
    zrdram = nc.dram_tensor("zrscratch", [B_PER_CORE, NH, N], F32)

    with tile.TileContext(nc) as tc:
        with (
            tc.tile_pool(name="const", bufs=1) as const,
            tc.tile_pool(name="work", bufs=2) as work,
            tc.tile_pool(name="epool", bufs=8) as epool,
            tc.tile_pool(name="ibpool", bufs=3) as ibpool,
            tc.tile_pool(name="small", bufs=2) as small,
            tc.tile_pool(name="psum", bufs=1, space="PSUM") as psum,
        ):
            # ---- PSUM arena: four [128, 1024] half-slots (2 banks each) ----
            arena = psum.tile([128, 4, N], F32, name="arena")
            slot_ctr = [0]

            def take_slots(k):
                s = []
                for _ in range(k):
                    s.append(slot_ctr[0] % 4)
                    slot_ctr[0] += 1
                return s

            # ---- persistent constants -----------------------------------
            w_sb = {}

            def load_weights(names):
                for nm in names:
                    w_sb[nm] = const.tile([128, 2, 2, C], FP8, name=f"w_{nm}")
                    nc.sync.dma_start(out=w_sb[nm], in_=w_ext[nm].ap())

            vecs_sb = const.tile([128, 4, CT], F32)
            nc.sync.dma_start(out=vecs_sb, in_=vecs_ext.ap())
            nsc_sb = vecs_sb[:, 0, :]
            nbi_sb = vecs_sb[:, 1, :]
            qb_sb = vecs_sb[:, 2, :]
            pb_sb = vecs_sb[:, 3, :]
            selr_sb = const.tile([128, CT, GROUPS], BF16)
            nc.sync.dma_start(out=selr_sb, in_=selr_ext.ap())
            sele_sb = const.tile([GROUPS, CT, 128], BF16)
            nc.sync.dma_start(out=sele_sb, in_=sele_ext.ap())
            eps_t = const.tile([GROUPS, 1], F32)
            nc.vector.memset(eps_t, EPS)
            esh_t = const.tile([128, 1], F32)
            nc.vector.memset(esh_t, EXP_SHIFT)
            # warm-keeper operands: tiny matmuls into dead arena rows keep the
            # PE's HAM activity monitor from re-throttling the clock to 1.2GHz
            # during short dependency stalls.
            wk_t = const.tile([128, 512], BF16)
            nc.gpsimd.memset(wk_t, 0.0)

            def warm(n, slot):
                for _ in range(n):
                    nc.tensor.matmul(
                        out=arena[96:128, slot, 0:512],
                        lhsT=wk_t[:, 0:32],
                        rhs=wk_t,
                        start=True,
                        stop=True,
                        tile_position=(0, 96),
                    )

            st = {0: {}, 1: {}}

            # ---- fill-unit queue (paired emission keeps slot alignment) --
            fillq = []

            def fill_pair():
                for _ in range(2):
                    if fillq:
                        fillq.pop(0)()

            # ---- x load --------------------------------------------------
            def emit_x_load(b):
                x_sb = work.tile([128, CT, N], F32, tag="x", name=f"x{b}")
                st[b]["x"] = x_sb
                for ct in range(CT):
                    nc.sync.dma_start(out=x_sb[:, ct, :], in_=x_ext.ap()[b][:, ct, :])

            # ---- GroupNorm stats -> csr/cb2 ------------------------------
            def emit_stats(b):
                x_sb = st[b]["x"]
                cstats = small.tile([128, CT, 2, 6], F32, tag="cstats")
                for ct in range(CT):
                    for sg in range(2):
                        nc.vector.bn_stats(
                            out=cstats[:, ct, sg, :],
                            in_=x_sb[:, ct, sg * 512 : (sg + 1) * 512],
                        )
                # per-(channel, chunk): a = mean_e+mean_o, bvar = cnt*var sums,
                # c2 = mean_e^2+mean_o^2; group-reduce via selector matmul.
                prep = small.tile([128, CT, 2, 3], F32, tag="prep")
                nc.vector.tensor_add(
                    out=prep[:, :, :, 0], in0=cstats[:, :, :, 1], in1=cstats[:, :, :, 4]
                )
                nc.vector.tensor_add(
                    out=prep[:, :, :, 1], in0=cstats[:, :, :, 2], in1=cstats[:, :, :, 5]
                )
                nc.vector.scalar_tensor_tensor(
                    out=cstats[:, :, :, 0], in0=cstats[:, :, :, 1], scalar=0.0,
                    in1=cstats[:, :, :, 1], op0=OP.add, op1=OP.mult,
                )
                nc.vector.scalar_tensor_tensor(
                    out=cstats[:, :, :, 3], in0=cstats[:, :, :, 4], scalar=0.0,
                    in1=cstats[:, :, :, 4], op0=OP.add, op1=OP.mult,
                )
                nc.vector.tensor_add(
                    out=prep[:, :, :, 2], in0=cstats[:, :, :, 0], in1=cstats[:, :, :, 3]
                )
                cb16 = small.tile([128, CT, 2, 3], BF16, tag="cb16")
                nc.vector.tensor_copy(out=cb16, in_=prep)
                ss = take_slots(2)
                gps = arena[0:GROUPS, ss[0], 0:6]
                for ct in range(CT):
                    nc.tensor.matmul(
                        out=gps,
                        lhsT=selr_sb[:, ct, :],
                        rhs=cb16[:, ct, :, :].rearrange("p s f -> p (s f)"),
                        start=(ct == 0),
                        stop=(ct == CT - 1),
                    )
                gsb = small.tile([GROUPS, 6], F32, tag="gsb")
                nc.vector.tensor_copy(out=gsb, in_=gps)
                gmv = small.tile([GROUPS, 4], F32, tag="gmv")
                nc.vector.tensor_add(out=gmv[:, 0:3], in0=gsb[:, 0:3], in1=gsb[:, 3:6])
                nc.vector.scalar_tensor_tensor(
                    out=gmv[:, 1:2], in0=gmv[:, 1:2], scalar=1.0 / 256.0,
                    in1=gmv[:, 2:3], op0=OP.mult, op1=OP.add,
                )
                nc.vector.scalar_tensor_tensor(
                    out=gmv[:, 3:4], in0=gmv[:, 0:1], scalar=0.0,
                    in1=gmv[:, 0:1], op0=OP.add, op1=OP.mult,
                )
                nc.vector.tensor_sub(out=gmv[:, 1:2], in0=gmv[:, 1:2], in1=gmv[:, 3:4])
                # rstd = exp(-0.5 * ln(var + eps)); Ln/Exp share one ACT table set
                lnv = small.tile([GROUPS, 1], F32, tag="lnv")
                nc.scalar.activation(out=lnv, in_=gmv[:, 1:2], func=AF.Ln, bias=eps_t)
                nc.scalar.activation(out=gmv[:, 1:2], in_=lnv, func=AF.Exp, scale=-0.5)
                gm16 = small.tile([GROUPS, 2], BF16, tag="gm16")
                nc.vector.tensor_copy(out=gm16, in_=gmv[:, 0:2])
                cps = arena[:, ss[1], 0:8]
                for ct in range(CT):
                    nc.tensor.matmul(
                        out=cps[:, ct * 2 : ct * 2 + 2],
                        lhsT=sele_sb[:, ct, :],
                        rhs=gm16,
                        start=True,
                        stop=True,
                    )
                cmv = cps.rearrange("p (ct s) -> p ct s", s=2)
                csr = small.tile([128, CT], F32, tag="csr", name=f"csr{b}")
                nc.vector.tensor_mul(out=csr, in0=cmv[:, :, 1], in1=nsc_sb)
                cb2 = small.tile([128, CT], F32, tag="cb2", name=f"cb2{b}")
                nc.vector.tensor_mul(out=cb2, in0=cmv[:, :, 0], in1=csr)
                nc.vector.tensor_sub(out=cb2, in0=nbi_sb, in1=cb2)
                st[b]["csr"] = csr
                st[b]["cb2"] = cb2

            # ---- h apply (gpsimd): x -> h fp8 channel-packed -------------
            def emit_h(b):
                h8 = work.tile([128, 2, 2, N], FP8, tag="h8", name=f"h8_{b}")
                st[b]["h8"] = h8
                csr, cb2 = st[b]["csr"], st[b]["cb2"]
                x_sb = st[b]["x"]
                for t in range(2):
                    for j in range(2):
                        ct = 2 * t + j
                        nc.gpsimd.tensor_scalar(
                            out=h8[:, t, j, :], in0=x_sb[:, ct, :],
                            scalar1=csr[:, ct : ct + 1], scalar2=cb2[:, ct : ct + 1],
                            op0=OP.mult, op1=OP.add,
                        )

            # ---- convs (fp8 DoubleRow) -----------------------------------
            def prep_conv(b):
                q_sb = work.tile([128, CT, N], BF16, tag="q", name=f"q{b}")
                k_sb = work.tile([128, CT, N], BF16, tag="k", name=f"k{b}")
                vt_sb = work.tile([128, NT // 2, 2, NH, VG], FP8, tag="vt", name=f"vt{b}")
                st[b].update({"q": q_sb, "k": k_sb, "vt": vt_sb})
                nc.vector.memset(vt_sb[:, :, :, :, D : D + 1], 1.0)

            def conv_qk_unit(b, nm, ct):
                def emit():
                    h8 = st[b]["h8"]
                    dst = st[b][nm]
                    s = take_slots(1)[0]
                    for ch in range(2):
                        for t in range(2):
                            nc.tensor.matmul(
                                out=arena[:, s, ch * 512 : (ch + 1) * 512],
                                lhsT=w_sb[nm][:, t, :, ct * 128 : (ct + 1) * 128],
                                rhs=h8[:, t, :, ch * 512 : (ch + 1) * 512],
                                start=(t == 0),
                                stop=(t == 1),
                                perf_mode=mybir.MatmulPerfMode.DoubleRow,
                            )
                    src = arena[:, s, :]
                    if nm == "q" and has_qb:
                        if b == 0:
                            nc.scalar.activation(
                                out=dst[:, ct, :], in_=src, func=AF.Identity,
                                bias=qb_sb[:, ct : ct + 1],
                            )
                        else:
                            nc.vector.tensor_scalar(
                                out=dst[:, ct, :], in0=src,
                                scalar1=qb_sb[:, ct : ct + 1], scalar2=None, op0=OP.add,
                            )
                    else:
                        if b == 0:
                            nc.scalar.copy(out=dst[:, ct, :], in_=src)
                        else:
                            nc.vector.tensor_copy(out=dst[:, ct, :], in_=src)
                return emit

            def conv_v_unit(b, ntp):
                def emit():
                    h8 = st[b]["h8"]
                    vt_sb = st[b]["vt"]
                    s = take_slots(1)[0]
                    for k in range(2):
                        nt = 2 * ntp + k
                        for t in range(2):
                            nc.tensor.matmul(
                                out=arena[:, s, k * 512 : (k + 1) * 512],
                                lhsT=h8[:, t, :, nt * 128 : (nt + 1) * 128],
                                rhs=w_sb["v"][:, t, :, :],
                                start=(t == 0),
                                stop=(t == 1),
                                perf_mode=mybir.MatmulPerfMode.DoubleRow,
                            )
                    nc.vector.tensor_copy(
                        out=vt_sb[:, ntp, :, :, 0:D],
                        in_=arena[:, s, :].rearrange("p (k h d) -> p k h d", k=2, d=D),
                    )
                return emit

            # ---- attention -----------------------------------------------
            def prep_att(b):
                att8 = work.tile([128, 2, 2, N], FP8, tag="att8", name=f"att8_{b}")
                st[b].update({"attz": {}, "att8": att8, "e": {}, "eb": {}})

            def emit_S(b, hp, mt):
                q_sb, k_sb = st[b]["q"], st[b]["k"]
                is_dve = (mt // 2) in DVE_PAIRS[hp]
                if mt % 2 == 0 and not is_dve:
                    e_t = epool.tile([128, 2, 2, N], FP8, tag="e", name=f"e{b}_{hp}_{mt // 2}")
                    st[b]["e"][(hp, mt // 2)] = e_t
                ss = take_slots(2)
                for ch in range(2):
                    for hi, p0 in ((0, 0), (1, 64)):
                        nc.tensor.matmul(
                            out=arena[:, ss[hi], ch * 512 : (ch + 1) * 512],
                            lhsT=k_sb[p0 : p0 + D, hp, mt * 128 : (mt + 1) * 128],
                            rhs=q_sb[p0 : p0 + D, hp, ch * 512 : (ch + 1) * 512],
                            start=True,
                            stop=True,
                            tile_position=(p0, 0),
                        )
                aligned = ss[0] % 2 == 0 and ss[1] == ss[0] + 1
                if is_dve:
                    eb = ibpool.tile([128, 2, N], I16, tag="eb", name=f"eb{b}_{hp}_{mt}")
                    st[b]["eb"][(hp, mt)] = eb
                    if aligned:
                        nc.vector.tensor_scalar(
                            out=eb, in0=arena[:, ss[0] : ss[0] + 2, :],
                            scalar1=SCHRA_A16, scalar2=SCHRA_B16, op0=OP.mult, op1=OP.add,
                        )
                    else:
                        for hi in range(2):
                            nc.vector.tensor_scalar(
                                out=eb[:, hi, :], in0=arena[:, ss[hi], :],
                                scalar1=SCHRA_A16, scalar2=SCHRA_B16, op0=OP.mult, op1=OP.add,
                            )
                else:
                    e_dst = st[b]["e"][(hp, mt // 2)][:, mt % 2, :, :]
                    if aligned:
                        nc.scalar.activation(
                            out=e_dst, in_=arena[:, ss[0] : ss[0] + 2, :],
                            func=AF.Exp, scale=0.125, bias=esh_t,
                        )
                    else:
                        for hi in range(2):
                            nc.scalar.activation(
                                out=e_dst[:, hi, :], in_=arena[:, ss[hi], :],
                                func=AF.Exp, scale=0.125, bias=esh_t,
                            )

            def emit_AV(b, hp):
                vt_sb = st[b]["vt"]
                attz = work.tile([D + 1, 2, N], F32, tag="attz", bufs=3, name=f"attz{b}_{hp}")
                st[b]["attz"][hp] = attz
                ss = take_slots(2)
                for hi in range(2):
                    h_ = 2 * hp + hi
                    pso = arena[0 : D + 1, ss[hi], :]
                    for ch in range(2):
                        steps = []
                        for mtp in range(NT // 2):
                            if mtp in DVE_PAIRS[hp]:
                                steps.append(("p", mtp, 0))
                                steps.append(("p", mtp, 1))
                            else:
                                steps.append(("dr", mtp, 0))
                        for i, (kind, mtp, j) in enumerate(steps):
                            if kind == "dr":
                                nc.tensor.matmul(
                                    out=pso[:, ch * 512 : (ch + 1) * 512],
                                    lhsT=vt_sb[:, mtp, :, h_, 0 : D + 1],
                                    rhs=st[b]["e"][(hp, mtp)][:, :, hi, ch * 512 : (ch + 1) * 512],
                                    start=(i == 0),
                                    stop=(i == len(steps) - 1),
                                    perf_mode=mybir.MatmulPerfMode.DoubleRow,
                                )
                            else:
                                eb = st[b]["eb"][(hp, 2 * mtp + j)]
                                nc.tensor.matmul(
                                    out=pso[:, ch * 512 : (ch + 1) * 512],
                                    lhsT=vt_sb[:, mtp, j, h_, 0 : D + 1],
                                    rhs=eb.bitcast(BF16)[:, hi, ch * 512 : (ch + 1) * 512],
                                    start=(i == 0),
                                    stop=(i == len(steps) - 1),
                                )
                # single copy for both heads (slots are an aligned pair);
                # Z rows (partition 64) ride along
                if ss[1] == ss[0] + 1:
                    nc.vector.tensor_copy(
                        out=attz, in_=arena[0 : D + 1, ss[0] : ss[0] + 2, :]
                    )
                else:
                    for hi in range(2):
                        nc.vector.tensor_copy(
                            out=attz[:, hi, :], in_=arena[0 : D + 1, ss[hi], :]
                        )
                warm(N_WARM_BOUNDARY, ss[0])

            def emit_zchain(b, hps):
                for hp in hps:
                    nc.gpsimd.dma_start(
                        out=zdram.ap()[b][2 * hp : 2 * hp + 2],
                        in_=st[b]["attz"][hp][D : D + 1, :, :],
                    )
                nrow = 2 * len(hps)
                zr = small.tile([4, N], F32, tag="zr")
                nc.sync.dma_start(
                    out=zr[0:nrow, :], in_=zdram.ap()[b][2 * hps[0] : 2 * hps[0] + nrow]
                )
                nc.vector.reciprocal_approx_fast(out=zr[0:nrow, :], in_=zr[0:nrow, :])
                nc.sync.dma_start(
                    out=zrdram.ap()[b][2 * hps[0] : 2 * hps[0] + nrow], in_=zr[0:nrow, :]
                )

            def emit_norm(b, hp):
                attz, att8 = st[b]["attz"][hp], st[b]["att8"]
                for hi in range(2):
                    h_ = 2 * hp + hi
                    rzb = small.tile([D, N], F32, tag="rzb", name=f"rzb{b}_{h_}")
                    r1 = zrdram.ap()[b][h_]  # [N]
                    src = bass.AP(
                        tensor=r1.tensor,
                        offset=r1.offset,
                        ap=[[0, D], list(r1.ap[0])],
                    )
                    nc.sync.dma_start(out=rzb, in_=src)
                    # gpsimd cores are partition-hardwired: only the hi=0 mul
                    # keeps in/out on the same partitions, so hi=1 stays on DVE
                    eng = nc.gpsimd if hi == 0 else nc.vector
                    eng.tensor_mul(
                        out=att8[64 * hi : 64 * hi + 64, hp // 2, hp % 2, :],
                        in0=attz[0:D, hi, :],
                        in1=rzb,
                    )

            # ---- proj + residual + store ---------------------------------
            def proj_unit(b, ct):
                def emit():
                    att8 = st[b]["att8"]
                    x_sb = st[b]["x"]
                    s = take_slots(1)[0]
                    for ch in range(2):
                        for t in range(2):
                            nc.tensor.matmul(
                                out=arena[:, s, ch * 512 : (ch + 1) * 512],
                                lhsT=w_sb["p"][:, t, :, ct * 128 : (ct + 1) * 128],
                                rhs=att8[:, t, :, ch * 512 : (ch + 1) * 512],
                                start=(t == 0),
                                stop=(t == 1),
                                perf_mode=mybir.MatmulPerfMode.DoubleRow,
                            )
                    nc.vector.scalar_tensor_tensor(
                        out=x_sb[:, ct, :], in0=arena[:, s, :],
                        scalar=pb_sb[:, ct : ct + 1] if has_pb else 0.0,
                        in1=x_sb[:, ct, :], op0=OP.add, op1=OP.add,
                    )
                    nc.sync.dma_start(out=out_ext.ap()[b][:, ct, :], in_=x_sb[:, ct, :])
                return emit

            # =========================== schedule =========================
            emit_x_load(0)
            load_weights(("q", "k"))
            emit_stats(0)
            load_weights(("v", "p"))
            emit_h(0)
            prep_conv(0)
            warm(N_WARM_PROLOGUE, 2)
            for ct in range(CT):
                conv_qk_unit(0, "q", ct)()
                conv_qk_unit(0, "k", ct)()
            for ntp in range(NT // 2):
                conv_v_unit(0, ntp)()
            emit_x_load(1)
            emit_stats(1)
            emit_h(1)
            prep_conv(1)
            for ct in range(CT):
                fillq.append(conv_qk_unit(1, "q", ct))
                fillq.append(conv_qk_unit(1, "k", ct))
            for ntp in range(NT // 2):
                fillq.append(conv_v_unit(1, ntp))
            prep_att(0)

            # attention b0; fill = b1 convs. AV(hp-1) is emitted right after
            # S(hp, 0) so the next head-pair's exp stream starts without
            # waiting behind the AV chains.
            for hp in range(4):
                for mt in range(NT):
                    emit_S(0, hp, mt)
                    if mt == 0 and hp > 0:
                        emit_AV(0, hp - 1)
                        if hp == 2:
                            emit_zchain(0, (0, 1))
                        if hp == 3:
                            emit_zchain(0, (2,))
                    if hp == 2 and mt == 3:
                        emit_norm(0, 0)
                    if hp == 2 and mt == 5:
                        emit_norm(0, 1)
                    if hp == 3 and mt == 3:
                        emit_norm(0, 2)
                    if hp < 3 and mt in (3, 6):
                        fill_pair()
            emit_AV(0, 3)
            emit_zchain(0, (3,))
            prep_att(1)

            # attention b1; fill = b0 proj
            for hp in range(4):
                for mt in range(NT):
                    emit_S(1, hp, mt)
                    if mt == 0 and hp > 0:
                        emit_AV(1, hp - 1)
                        if hp == 2:
                            emit_zchain(1, (0, 1))
                        if hp == 3:
                            emit_zchain(1, (2,))
                    if hp == 0 and mt == 1:
                        emit_norm(0, 3)
                    if hp == 0 and mt == 3:
                        fillq.extend([proj_unit(0, 0), proj_unit(0, 1)])
                        fill_pair()
                    if hp == 0 and mt == 6:
                        fillq.extend([proj_unit(0, 2), proj_unit(0, 3)])
                        fill_pair()
                    if hp == 2 and mt == 3:
                        emit_norm(1, 0)
                    if hp == 2 and mt == 5:
                        emit_norm(1, 1)
                    if hp == 3 and mt == 3:
                        emit_norm(1, 2)
            emit_AV(1, 3)
            emit_zchain(1, (3,))
            emit_norm(1, 3)
            for ct in range(CT):
                proj_unit(1, ct)()

    nc.compile()
    return nc


def kernel(x, norm_scale, norm_bias, q_w, q_b, k_w, k_b, v_w, v_b, proj_w, proj_b):
    import ml_dtypes

    fp8 = ml_dtypes.float8_e4m3fn
    bf16 = ml_dtypes.bfloat16

    x = np.asarray(x, dtype=np.float32)
    b, c, hh, ww = x.shape
    assert (b, c, hh * ww) == (16, C, N)
    xr = np.ascontiguousarray(x.reshape(b, CT, 128, hh * ww).transpose(0, 2, 1, 3))

    def _w8(w):
        wT = np.asarray(w, np.float32).T  # [Cin, Cout]
        return np.ascontiguousarray(
            wT.reshape(2, 2, 128, C).transpose(2, 0, 1, 3).astype(fp8)
        )

    pb_eff = np.asarray(proj_b, np.float32) + np.asarray(proj_w, np.float32) @ np.asarray(
        v_b, np.float32
    )
    vecs = np.stack(
        [
            np.asarray(v, np.float32).reshape(CT, 128).T
            for v in (norm_scale, norm_bias, q_b, pb_eff)
        ],
        axis=1,
    )  # [128, 4, CT]
    selr = np.zeros((128, CT, GROUPS), np.float32)
    sele = np.zeros((GROUPS, CT, 128), np.float32)
    for ct in range(CT):
        for p in range(128):
            g = ct * 8 + p // GS
            selr[p, ct, g] = 1.0 / 64.0
            sele[g, ct, p] = 1.0

    wts = {
        "qw8": _w8(q_w),
        "kw8": _w8(k_w),
        "vw8": _w8(v_w),
        "pw8": _w8(proj_w),
        "vecs": np.ascontiguousarray(vecs),
        "selr": np.ascontiguousarray(selr.astype(bf16)),
        "sele": np.ascontiguousarray(sele.astype(bf16)),
    }
    has_qb = bool(np.any(np.asarray(q_b)))
    has_pb = bool(np.any(pb_eff))

    nc = build_nc(has_qb, has_pb)
    in_maps = []
    for i in range(N_CORES):
        m = dict(wts)
        m["x"] = np.ascontiguousarray(xr[i * B_PER_CORE : (i + 1) * B_PER_CORE])
        in_maps.append(m)

    res = run_bass_kernel_spmd(nc, in_maps, core_ids=list(range(N_CORES)))
    kernel.last_result = res
    out = np.concatenate([res.results[i]["out"] for i in range(N_CORES)], axis=0)
    out = out.transpose(0, 2, 1, 3).reshape(b, c, hh, ww)
    return np.ascontiguousarray(out).astype(np.float32)


# revision 30
# speedup vs baseline: 1.2587x; 1.2587x over previous
"""Trainium2 Bass kernel for an AttentionBlock (GroupNorm + 8-head attention
over 32x32 spatial + proj + residual), data-parallel over batch on 8 cores.

Per batch element (x: [C=512, n=1024]):
  h   = GroupNorm32(x) * scale + bias          (h stored fp8, channel-packed)
  q,k = Wq h, Wk h  (bf16; k-bias provably drops out of softmax; q-bias kept)
  v^T = h^T Wv^T    (fp8, n-major, DoubleRow-packed, + ones row for Z)
  S^T = k^T q / 8   per head (bf16 matmuls, two heads packed in PE quadrants)
  e   = exp(S/8 - 2.5)  -> fp8   [ACT engine big chunks + DVE Schraudolph share]
  o   = v e (fp8 DoubleRow), Z from the ones row; att = o / Z
  y   = x + Wp att + pb_eff      (pb_eff = pb + Wp @ vb, folded host-side)

All convs run fp8 DoubleRow with channel-pair packing (c = 256t + 128j + p).
PSUM is managed as one 8-bank arena of four [128,1024] half-slots.
"""

import numpy as np

import concourse.bacc as bacc
import concourse.bass as bass
import concourse.tile as tile
from concourse import mybir
from concourse.bass_utils import run_bass_kernel_spmd

F32 = mybir.dt.float32
I32 = mybir.dt.int32
BF16 = mybir.dt.bfloat16
FP8 = mybir.dt.float8e4
AF = mybir.ActivationFunctionType
OP = mybir.AluOpType

C = 512
NH = 8
D = 64
N = 1024
GROUPS = 32
GS = C // GROUPS
EPS = 1e-5
B_PER_CORE = 2
N_CORES = 8

CT = 4    # channel tiles of 128
NT = 8    # n tiles of 128
VG = 66   # vT per-head group stride (64 data + 1 ones + 1 pad)

LOG2E = 1.4426950408889634
EXP_SHIFT = -2.5          # softmax-invariant logit shift keeps e in fp8 range
# Schraudolph exp in bf16 bit domain: int16 bits = round(S*A16 + B16) is the
# bf16 pattern of ~exp(0.125*S + EXP_SHIFT) (3% approx err; softmax
# normalization cancels the common-mode part). A@V reads the bits via bitcast.
SCHRA_A16 = float((1 << 7) * LOG2E * 0.125)
SCHRA_B16 = float((1 << 7) * (127.0 - 0.0450466 + EXP_SHIFT * LOG2E))

# e-tile pairs (of n-tiles) per head-pair whose exp runs on DVE instead of ACT
DVE_PAIRS = {0: (1,), 1: (1,), 2: (1,), 3: ()}
I16 = mybir.dt.int16
N_WARM_BOUNDARY = 0   # warm-keeper dummy matmuls after each AV block
N_WARM_PROLOGUE = 0


def build_nc(has_qb, has_pb, trace_sim=False):
    nc = bacc.Bacc()

    x_ext = nc.declare_dram_parameter("x", [B_PER_CORE, 128, CT, N], F32, isOutput=False)
    w_ext = {}
    for nm in ("q", "k", "v", "p"):
        w_ext[nm] = nc.declare_dram_parameter(f"{nm}w8", [128, 2, 2, C], FP8, isOutput=False)
    # packed per-channel vectors: [128, 4, CT] = (norm_scale, norm_bias, qb, pb_eff)
    vecs_ext = nc.declare_dram_parameter("vecs", [128, 4, CT], F32, isOutput=False)
    selr_ext = nc.declare_dram_parameter("selr", [128, CT, GROUPS], BF16, isOutput=False)
    sele_ext = nc.declare_dram_parameter("sele", [GROUPS, CT, 128], BF16, isOutput=False)
    out_ext = nc.declare_dram_parameter("out", [B_PER_CORE, 128, CT, N], F32, isOutput=True)

    zdram = nc.dram_tensor("zscratch", [B_PER_CORE, NH, N], F32)
    zrdram = nc.dram_tensor("zrscratch", [B_PER_CORE, NH, N], F32)

    with tile.TileContext(nc) as tc:
        with (
            tc.tile_pool(name="const", bufs=1) as const,
            tc.tile_pool(name="work", bufs=2) as work,
            tc.tile_pool(name="epool", bufs=8) as epool,
            tc.tile_pool(name="ibpool", bufs=3) as ibpool,
            tc.tile_pool(name="small", bufs=2) as small,
            tc.tile_pool(name="psum", bufs=1, space="PSUM") as psum,
        ):
            # ---- PSUM arena: four [128, 1024] half-slots (2 banks each) ----
            arena = psum.tile([128, 4, N], F32, name="arena")
            slot_ctr = [0]

            def take_slots(k):
                s = []
                for _ in range(k):
                    s.append(slot_ctr[0] % 4)
                    slot_ctr[0] += 1
                return s

            # ---- persistent constants -----------------------------------
            w_sb = {}

            def load_weights(names):
                for nm in names:
                    w_sb[nm] = const.tile([128, 2, 2, C], FP8, name=f"w_{nm}")
                    nc.sync.dma_start(out=w_sb[nm], in_=w_ext[nm].ap())

            vecs_sb = const.tile([128, 4, CT], F32)
            nc.sync.dma_start(out=vecs_sb, in_=vecs_ext.ap())
            nsc_sb = vecs_sb[:, 0, :]
            nbi_sb = vecs_sb[:, 1, :]
            qb_sb = vecs_sb[:, 2, :]
            pb_sb = vecs_sb[:, 3, :]
            selr_sb = const.tile([128, CT, GROUPS], BF16)
            nc.sync.dma_start(out=selr_sb, in_=selr_ext.ap())
            sele_sb = const.tile([GROUPS, CT, 128], BF16)
            nc.sync.dma_start(out=sele_sb, in_=sele_ext.ap())
            eps_t = const.tile([GROUPS, 1], F32)
            nc.vector.memset(eps_t, EPS)
            esh_t = const.tile([128, 1], F32)
            nc.vector.memset(esh_t, EXP_SHIFT)
            # warm-keeper operands: tiny matmuls into dead arena rows keep the
            # PE's HAM activity monitor from re-throttling the clock to 1.2GHz
            # during short dependency stalls.
            wk_t = const.tile([128, 512], BF16)
            nc.gpsimd.memset(wk_t, 0.0)

            def warm(n, slot):
                for _ in range(n):
                    nc.tensor.matmul(
                        out=arena[96:128, slot, 0:512],
                        lhsT=wk_t[:, 0:32],
                        rhs=wk_t,
                        start=True,
                        stop=True,
                        tile_position=(0, 96),
                    )

            st = {0: {}, 1: {}}

            # ---- fill-unit queue (paired emission keeps slot alignment) --
            fillq = []

            def fill_pair():
                for _ in range(2):
                    if fillq:
                        fillq.pop(0)()

            # ---- x load --------------------------------------------------
            def emit_x_load(b):
                x_sb = work.tile([128, CT, N], F32, tag="x", name=f"x{b}")
                st[b]["x"] = x_sb
                for ct in range(CT):
                    nc.sync.dma_start(out=x_sb[:, ct, :], in_=x_ext.ap()[b][:, ct, :])

            # ---- GroupNorm stats -> csr/cb2 ------------------------------
            def emit_stats(b):
                x_sb = st[b]["x"]
                cstats = small.tile([128, CT, 2, 6], F32, tag="cstats")
                for ct in range(CT):
                    for sg in range(2):
                        nc.vector.bn_stats(
                            out=cstats[:, ct, sg, :],
                            in_=x_sb[:, ct, sg * 512 : (sg + 1) * 512],
                        )
                # per-(channel, chunk): a = mean_e+mean_o, bvar = cnt*var sums,
                # c2 = mean_e^2+mean_o^2; group-reduce via selector matmul.
                prep = small.tile([128, CT, 2, 3], F32, tag="prep")
                nc.vector.tensor_add(
                    out=prep[:, :, :, 0], in0=cstats[:, :, :, 1], in1=cstats[:, :, :, 4]
                )
                nc.vector.tensor_add(
                    out=prep[:, :, :, 1], in0=cstats[:, :, :, 2], in1=cstats[:, :, :, 5]
                )
                nc.vector.scalar_tensor_tensor(
                    out=cstats[:, :, :, 0], in0=cstats[:, :, :, 1], scalar=0.0,
                    in1=cstats[:, :, :, 1], op0=OP.add, op1=OP.mult,
                )
                nc.vector.scalar_tensor_tensor(
                    out=cstats[:, :, :, 3], in0=cstats[:, :, :, 4], scalar=0.0,
                    in1=cstats[:, :, :, 4], op0=OP.add, op1=OP.mult,
                )
                nc.vector.tensor_add(
                    out=prep[:, :, :, 2], in0=cstats[:, :, :, 0], in1=cstats[:, :, :, 3]
                )
                cb16 = small.tile([128, CT, 2, 3], BF16, tag="cb16")
                nc.vector.tensor_copy(out=cb16, in_=prep)
                ss = take_slots(2)
                gps = arena[0:GROUPS, ss[0], 0:6]
                for ct in range(CT):
                    nc.tensor.matmul(
                        out=gps,
                        lhsT=selr_sb[:, ct, :],
                        rhs=cb16[:, ct, :, :].rearrange("p s f -> p (s f)"),
                        start=(ct == 0),
                        stop=(ct == CT - 1),
                    )
                gsb = small.tile([GROUPS, 6], F32, tag="gsb")
                nc.vector.tensor_copy(out=gsb, in_=gps)
                gmv = small.tile([GROUPS, 4], F32, tag="gmv")
                nc.vector.tensor_add(out=gmv[:, 0:3], in0=gsb[:, 0:3], in1=gsb[:, 3:6])
                nc.vector.scalar_tensor_tensor(
                    out=gmv[:, 1:2], in0=gmv[:, 1:2], scalar=1.0 / 256.0,
                    in1=gmv[:, 2:3], op0=OP.mult, op1=OP.add,
                )
                nc.vector.scalar_tensor_tensor(
                    out=gmv[:, 3:4], in0=gmv[:, 0:1], scalar=0.0,
                    in1=gmv[:, 0:1], op0=OP.add, op1=OP.mult,
                )
                nc.vector.tensor_sub(out=gmv[:, 1:2], in0=gmv[:, 1:2], in1=gmv[:, 3:4])
                # rstd = exp(-0.5 * ln(var + eps)); Ln/Exp share one ACT table set
                lnv = small.tile([GROUPS, 1], F32, tag="lnv")
                nc.scalar.activation(out=lnv, in_=gmv[:, 1:2], func=AF.Ln, bias=eps_t)
                nc.scalar.activation(out=gmv[:, 1:2], in_=lnv, func=AF.Exp, scale=-0.5)
                gm16 = small.tile([GROUPS, 2], BF16, tag="gm16")
                nc.vector.tensor_copy(out=gm16, in_=gmv[:, 0:2])
                cps = arena[:, ss[1], 0:8]
                for ct in range(CT):
                    nc.tensor.matmul(
                        out=cps[:, ct * 2 : ct * 2 + 2],
                        lhsT=sele_sb[:, ct, :],
                        rhs=gm16,
                        start=True,
                        stop=True,
                    )
                cmv = cps.rearrange("p (ct s) -> p ct s", s=2)
                csr = small.tile([128, CT], F32, tag="csr", name=f"csr{b}")
                nc.vector.tensor_mul(out=csr, in0=cmv[:, :, 1], in1=nsc_sb)
                cb2 = small.tile([128, CT], F32, tag="cb2", name=f"cb2{b}")
                nc.vector.tensor_mul(out=cb2, in0=cmv[:, :, 0], in1=csr)
                nc.vector.tensor_sub(out=cb2, in0=nbi_sb, in1=cb2)
                st[b]["csr"] = csr
                st[b]["cb2"] = cb2

            # ---- h apply (gpsimd): x -> h fp8 channel-packed -------------
            def emit_h(b):
                h8 = work.tile([128, 2, 2, N], FP8, tag="h8", name=f"h8_{b}")
                st[b]["h8"] = h8
                csr, cb2 = st[b]["csr"], st[b]["cb2"]
                x_sb = st[b]["x"]
                for t in range(2):
                    for j in range(2):
                        ct = 2 * t + j
                        nc.gpsimd.tensor_scalar(
                            out=h8[:, t, j, :], in0=x_sb[:, ct, :],
                            scalar1=csr[:, ct : ct + 1], scalar2=cb2[:, ct : ct + 1],
                            op0=OP.mult, op1=OP.add,
                        )

            # ---- convs (fp8 DoubleRow) -----------------------------------
            def prep_conv(b):
                q_sb = work.tile([128, CT, N], BF16, tag="q", name=f"q{b}")
                k_sb = work.tile([128, CT, N], BF16, tag="k", name=f"k{b}")
                vt_sb = work.tile([128, NT // 2, 2, NH, VG], FP8, tag="vt", name=f"vt{b}")
                st[b].update({"q": q_sb, "k": k_sb, "vt": vt_sb})
                nc.vector.memset(vt_sb[:, :, :, :, D : D + 1], 1.0)

            def conv_qk_unit(b, nm, ct):
                def emit():
                    h8 = st[b]["h8"]
                    dst = st[b][nm]
                    s = take_slots(1)[0]
                    for ch in range(2):
                        for t in range(2):
                            nc.tensor.matmul(
                                out=arena[:, s, ch * 512 : (ch + 1) * 512],
                                lhsT=w_sb[nm][:, t, :, ct * 128 : (ct + 1) * 128],
                                rhs=h8[:, t, :, ch * 512 : (ch + 1) * 512],
                                start=(t == 0),
                                stop=(t == 1),
                                perf_mode=mybir.MatmulPerfMode.DoubleRow,
                            )
                    src = arena[:, s, :]
                    if nm == "q" and has_qb:
                        if b == 0:
                            nc.scalar.activation(
                                out=dst[:, ct, :], in_=src, func=AF.Identity,
                                bias=qb_sb[:, ct : ct + 1],
                            )
                        else:
                            nc.vector.tensor_scalar(
                                out=dst[:, ct, :], in0=src,
                                scalar1=qb_sb[:, ct : ct + 1], scalar2=None, op0=OP.add,
                            )
                    else:
                        if b == 0:
                            nc.scalar.copy(out=dst[:, ct, :], in_=src)
                        else:
                            nc.vector.tensor_copy(out=dst[:, ct, :], in_=src)
                return emit

            def conv_v_unit(b, ntp):
                def emit():
                    h8 = st[b]["h8"]
                    vt_sb = st[b]["vt"]
                    s = take_slots(1)[0]
                    for k in range(2):
                        nt = 2 * ntp + k
                        for t in range(2):
                            nc.tensor.matmul(
                                out=arena[:, s, k * 512 : (k + 1) * 512],
                                lhsT=h8[:, t, :, nt * 128 : (nt + 1) * 128],
                                rhs=w_sb["v"][:, t, :, :],
                                start=(t == 0),
                                stop=(t == 1),
                                perf_mode=mybir.MatmulPerfMode.DoubleRow,
                            )
                    nc.vector.tensor_copy(
                        out=vt_sb[:, ntp, :, :, 0:D],
                        in_=arena[:, s, :].rearrange("p (k h d) -> p k h d", k=2, d=D),
                    )
                return emit

            # ---- attention -----------------------------------------------
            def prep_att(b):
                att8 = work.tile([128, 2, 2, N], FP8, tag="att8", name=f"att8_{b}")
                st[b].update({"attz": {}, "att8": att8, "e": {}, "eb": {}})

            def emit_S(b, hp, mt):
                q_sb, k_sb = st[b]["q"], st[b]["k"]
                is_dve = (mt // 2) in DVE_PAIRS[hp]
                if mt % 2 == 0 and not is_dve:
                    e_t = epool.tile([128, 2, 2, N], FP8, tag="e", name=f"e{b}_{hp}_{mt // 2}")
                    st[b]["e"][(hp, mt // 2)] = e_t
                ss = take_slots(2)
                for ch in range(2):
                    for hi, p0 in ((0, 0), (1, 64)):
                        nc.tensor.matmul(
                            out=arena[:, ss[hi], ch * 512 : (ch + 1) * 512],
                            lhsT=k_sb[p0 : p0 + D, hp, mt * 128 : (mt + 1) * 128],
                            rhs=q_sb[p0 : p0 + D, hp, ch * 512 : (ch + 1) * 512],
                            start=True,
                            stop=True,
                            tile_position=(p0, 0),
                        )
                aligned = ss[0] % 2 == 0 and ss[1] == ss[0] + 1
                if is_dve:
                    eb = ibpool.tile([128, 2, N], I16, tag="eb", name=f"eb{b}_{hp}_{mt}")
                    st[b]["eb"][(hp, mt)] = eb
                    if aligned:
                        nc.vector.tensor_scalar(
                            out=eb, in0=arena[:, ss[0] : ss[0] + 2, :],
                            scalar1=SCHRA_A16, scalar2=SCHRA_B16, op0=OP.mult, op1=OP.add,
                        )
                    else:
                        for hi in range(2):
                            nc.vector.tensor_scalar(
                                out=eb[:, hi, :], in0=arena[:, ss[hi], :],
                                scalar1=SCHRA_A16, scalar2=SCHRA_B16, op0=OP.mult, op1=OP.add,
                            )
                else:
                    e_dst = st[b]["e"][(hp, mt // 2)][:, mt % 2, :, :]
                    if aligned:
                        nc.scalar.activation(
                            out=e_dst, in_=arena[:, ss[0] : ss[0] + 2, :],
                            func=AF.Exp, scale=0.125, bias=esh_t,
                        )
                    else:
                        for hi in range(2):
                            nc.scalar.activation(
                                out=e_dst[:, hi, :], in_=arena[:, ss[hi], :],
                                func=AF.Exp, scale=0.125, bias=esh_t,
                            )

            def emit_AV(b, hp):
                vt_sb = st[b]["vt"]
                attz = work.tile([D + 1, 2, N], F32, tag="attz", bufs=3, name=f"attz{b}_{hp}")
                st[b]["attz"][hp] = attz
                ss = take_slots(2)
                for hi in range(2):
                    h_ = 2 * hp + hi
                    pso = arena[0 : D + 1, ss[hi], :]
                    for ch in range(2):
                        steps = []
                        for mtp in range(NT // 2):
                            if mtp in DVE_PAIRS[hp]:
                                steps.append(("p", mtp, 0))
                                steps.append(("p", mtp, 1))
                            else:
                                steps.append(("dr", mtp, 0))
                        for i, (kind, mtp, j) in enumerate(steps):
                            if kind == "dr":
                                nc.tensor.matmul(
                                    out=pso[:, ch * 512 : (ch + 1) * 512],
                                    lhsT=vt_sb[:, mtp, :, h_, 0 : D + 1],
                                    rhs=st[b]["e"][(hp, mtp)][:, :, hi, ch * 512 : (ch + 1) * 512],
                                    start=(i == 0),
                                    stop=(i == len(steps) - 1),
                                    perf_mode=mybir.MatmulPerfMode.DoubleRow,
                                )
                            else:
                                eb = st[b]["eb"][(hp, 2 * mtp + j)]
                                nc.tensor.matmul(
                                    out=pso[:, ch * 512 : (ch + 1) * 512],
                                    lhsT=vt_sb[:, mtp, j, h_, 0 : D + 1],
                                    rhs=eb.bitcast(BF16)[:, hi, ch * 512 : (ch + 1) * 512],
                                    start=(i == 0),
                                    stop=(i == len(steps) - 1),
                                )
                # single copy for both heads (slots are an aligned pair);
                # Z rows (partition 64) ride along
                if ss[1] == ss[0] + 1:
                    nc.vector.tensor_copy(
                        out=attz, in_=arena[0 : D + 1, ss[0] : ss[0] + 2, :]
                    )
                else:
                    for hi in range(2):
                        nc.vector.tensor_copy(
                            out=attz[:, hi, :], in_=arena[0 : D + 1, ss[hi], :]
                        )
                warm(N_WARM_BOUNDARY, ss[0])

            def emit_zchain(b, hps):
                for hp in hps:
                    nc.gpsimd.dma_start(
                        out=zdram.ap()[b][2 * hp : 2 * hp + 2],
                        in_=st[b]["attz"][hp][D : D + 1, :, :],
                    )
                nrow = 2 * len(hps)
                zr = small.tile([4, N], F32, tag="zr")
                nc.sync.dma_start(
                    out=zr[0:nrow, :], in_=zdram.ap()[b][2 * hps[0] : 2 * hps[0] + nrow]
                )
                nc.vector.reciprocal_approx_fast(out=zr[0:nrow, :], in_=zr[0:nrow, :])
                nc.sync.dma_start(
                    out=zrdram.ap()[b][2 * hps[0] : 2 * hps[0] + nrow], in_=zr[0:nrow, :]
                )

            def emit_norm(b, hp):
                attz, att8 = st[b]["attz"][hp], st[b]["att8"]
                for hi in range(2):
                    h_ = 2 * hp + hi
                    rzb = small.tile([D, N], F32, tag="rzb", name=f"rzb{b}_{h_}")
                    r1 = zrdram.ap()[b][h_]  # [N]
                    src = bass.AP(
                        tensor=r1.tensor,
                        offset=r1.offset,
                        ap=[[0, D], list(r1.ap[0])],
                    )
                    nc.sync.dma_start(out=rzb, in_=src)
                    # gpsimd cores are partition-hardwired: only the hi=0 mul
                    # keeps in/out on the same partitions, so hi=1 stays on DVE
                    eng = nc.gpsimd if hi == 0 else nc.vector
                    eng.tensor_mul(
                        out=att8[64 * hi : 64 * hi + 64, hp // 2, hp % 2, :],
                        in0=attz[0:D, hi, :],
                        in1=rzb,
                    )

            # ---- proj + residual + store ---------------------------------
            def proj_unit(b, ct):
                def emit():
                    att8 = st[b]["att8"]
                    x_sb = st[b]["x"]
                    s = take_slots(1)[0]
                    for ch in range(2):
                        for t in range(2):
                            nc.tensor.matmul(
                                out=arena[:, s, ch * 512 : (ch + 1) * 512],
                                lhsT=w_sb["p"][:, t, :, ct * 128 : (ct + 1) * 128],
                                rhs=att8[:, t, :, ch * 512 : (ch + 1) * 512],
                                start=(t == 0),
                                stop=(t == 1),
                                perf_mode=mybir.MatmulPerfMode.DoubleRow,
                            )
                    nc.vector.scalar_tensor_tensor(
                        out=x_sb[:, ct, :], in0=arena[:, s, :],
                        scalar=pb_sb[:, ct : ct + 1] if has_pb else 0.0,
                        in1=x_sb[:, ct, :], op0=OP.add, op1=OP.add,
                    )
                    nc.sync.dma_start(out=out_ext.ap()[b][:, ct, :], in_=x_sb[:, ct, :])
                return emit

            # =========================== schedule =========================
            emit_x_load(0)
            load_weights(("q", "k"))
            emit_stats(0)
            load_weights(("v", "p"))
            emit_h(0)
            prep_conv(0)
            warm(N_WARM_PROLOGUE, 2)
            for ct in range(CT):
                conv_qk_unit(0, "q", ct)()
                conv_qk_unit(0, "k", ct)()
            for ntp in range(NT // 2):
                conv_v_unit(0, ntp)()
            emit_x_load(1)
            emit_stats(1)
            emit_h(1)
            prep_conv(1)
            for ct in range(CT):
                fillq.append(conv_qk_unit(1, "q", ct))
                fillq.append(conv_qk_unit(1, "k", ct))
            for ntp in range(NT // 2):
                fillq.append(conv_v_unit(1, ntp))
            prep_att(0)

            # attention b0; fill = b1 convs. AV(hp-1) is emitted right after
            # S(hp, 0) so the next head-pair's exp stream starts without
            # waiting behind the AV chains.
            for hp in range(4):
                for mt in range(NT):
                    emit_S(0, hp, mt)
                    if mt == 0 and hp > 0:
                        emit_AV(0, hp - 1)
                        if hp == 2:
                            emit_zchain(0, (0, 1))
                        if hp == 3:
                            emit_zchain(0, (2,))
                    if hp == 2 and mt == 3:
                        emit_norm(0, 0)
                    if hp == 2 and mt == 5:
                        emit_norm(0, 1)
                    if hp == 3 and mt == 3:
                        emit_norm(0, 2)
                    if hp < 3 and mt in (3, 6):
                        fill_pair()
            emit_AV(0, 3)
            emit_zchain(0, (3,))
            prep_att(1)

            # attention b1; fill = b0 proj
            for hp in range(4):
                for mt in range(NT):
                    emit_S(1, hp, mt)
                    if mt == 0 and hp > 0:
                        emit_AV(1, hp - 1)
                        if hp == 2:
                            emit_zchain(1, (0, 1))
                        if hp == 3:
                            emit_zchain(1, (2,))
                    if hp == 0 and mt == 1:
                        emit_norm(0, 3)
                    if hp == 0 and mt == 3:
                        fillq.extend([proj_unit(0, 0), proj_unit(0, 1)])
                        fill_pair()
                    if hp == 0 and mt == 6:
                        fillq.extend([proj_unit(0, 2), proj_unit(0, 3)])
                        fill_pair()
                    if hp == 2 and mt == 3:
                        emit_norm(1, 0)
                    if hp == 2 and mt == 5:
                        emit_norm(1, 1)
                    if hp == 3 and mt == 3:
                        emit_norm(1, 2)
            emit_AV(1, 3)
            emit_zchain(1, (3,))
            emit_norm(1, 3)
            for ct in range(CT):
                proj_unit(1, ct)()

    nc.compile()
    return nc


def kernel(x, norm_scale, norm_bias, q_w, q_b, k_w, k_b, v_w, v_b, proj_w, proj_b):
    import ml_dtypes

    fp8 = ml_dtypes.float8_e4m3fn
    bf16 = ml_dtypes.bfloat16

    x = np.asarray(x, dtype=np.float32)
    b, c, hh, ww = x.shape
    assert (b, c, hh * ww) == (16, C, N)
    xr = np.ascontiguousarray(x.reshape(b, CT, 128, hh * ww).transpose(0, 2, 1, 3))

    def _w8(w):
        wT = np.asarray(w, np.float32).T  # [Cin, Cout]
        return np.ascontiguousarray(
            wT.reshape(2, 2, 128, C).transpose(2, 0, 1, 3).astype(fp8)
        )

    pb_eff = np.asarray(proj_b, np.float32) + np.asarray(proj_w, np.float32) @ np.asarray(
        v_b, np.float32
    )
    vecs = np.stack(
        [
            np.asarray(v, np.float32).reshape(CT, 128).T
            for v in (norm_scale, norm_bias, q_b, pb_eff)
        ],
        axis=1,
    )  # [128, 4, CT]
    selr = np.zeros((128, CT, GROUPS), np.float32)
    sele = np.zeros((GROUPS, CT, 128), np.float32)
    for ct in range(CT):
        for p in range(128):
            g = ct * 8 + p // GS
            selr[p, ct, g] = 1.0 / 64.0
            sele[g, ct, p] = 1.0

    wts = {
        "qw8": _w8(q_w),
        "kw8": _w8(k_w),
        "vw8": _w8(v_w),
        "pw8": _w8(proj_w),
        "vecs": np.ascontiguousarray(vecs),
        "selr": np.ascontiguousarray(selr.astype(bf16)),
        "sele": np.ascontiguousarray(sele.astype(bf16)),
    }
    has_qb = bool(np.any(np.asarray(q_b)))
    has_pb = bool(np.any(pb_eff))

    nc = build_nc(has_qb, has_pb)
    in_maps = []
    for i in range(N_CORES):
        m = dict(wts)
        m["x"] = np.ascontiguousarray(xr[i * B_PER_CORE : (i + 1) * B_PER_CORE])
        in_maps.append(m)

    res = run_bass_kernel_spmd(nc, in_maps, core_ids=list(range(N_CORES)))
    kernel.last_result = res
    out = np.concatenate([res.results[i]["out"] for i in range(N_CORES)], axis=0)
    out = out.transpose(0, 2, 1, 3).reshape(b, c, hh, ww)
    return np.ascontiguousarray(out).astype(np.float32)
